# revision 6
# baseline (speedup 1.0000x reference)
"""Trainium2 Bass kernel for nn_MultiHeadAttention_7834020348049.

Reference computation (per token, no cross-token interaction):
    qn  = LayerNorm(q) * gamma_m + beta_m
    kvn = LayerNorm(kv) * gamma_l + beta_l
    Q = qn @ Wq.T ; K,V = split(kvn @ Wkv.T)
    per token: scores[h,g] = Q[h,:] . K[g,:] / sqrt(128)  (8x8 over heads)
    ctx[h,:] = softmax_g(scores) @ V
    out = ctx @ Wo.T

Sharding: pure data-parallel over the 16*2048 = 32768 tokens -> 4096/core.

Per-core pipeline (bf16 matmuls, fp32 PSUM):
  token-major LN (bn_stats/bn_aggr, rsqrt via bit-trick+Newton on DVE)
  -> PE transpose to feature-major qn^T / kvn^T (bf16)
  -> Q/K/V projections, weights stationary, N=512 moving, outputs copied
     into head-major-within-16-token-subtile layout (contiguous 16-chunks)
  -> per 16-token sub-tile (partition index p = h*16 + t):
     scores matmul S[(h,t),(g,t')] (128x128), -60 off-diagonal-block mask
     added in PSUM (batched DVE), batched exp on ACT, row sums + recip +
     P=E/Z on DVE, PE transpose of P -> L, PE transpose of V slice -> vb,
     ctx^T = vb.T @ L
  -> token-major O-projection (ctx head-slices stationary), bf16 output.
ACT engine only ever runs Exp and Copy (single activation table set).
"""
import sys, os
sys.path.insert(0, "/opt/trn_rl_repo")
os.environ.setdefault("JAX_PLATFORMS", "cpu")

from contextlib import ExitStack
import numpy as np
import ml_dtypes

import concourse.bass as bass
import concourse.bacc as bacc
import concourse.tile as tile
from concourse import mybir
from concourse.masks import make_identity
from concourse.bass_utils import run_bass_kernel_spmd

F32 = mybir.dt.float32
BF16 = mybir.dt.bfloat16
I32 = mybir.dt.int32

DIM = 1024
HEADS = 8
DHEAD = 128
NCORES = 8

TT = 128          # tokens per tile (partition dim)
TC = 512          # tokens per chunk (projection moving-dim)
TS = 16           # tokens per attention sub-tile
TPC = TC // TT    # tiles per chunk (4)
SPT = TT // TS    # sub-tiles per tile (8)
SPC = TC // TS    # sub-tiles per chunk (32)
KT_F = DIM // 128 # k-tiles for the 1024-feature contraction (8)

MAGIC = 0x5F3759DF  # fast inverse sqrt seed


def build_nc(T, with_bias_q=False, with_bias_kv=False):
    """Build the single-core Bass program for T tokens."""
    nc = bacc.Bacc(trn_type="TRN2", target_bir_lowering=False)

    q_d = nc.dram_tensor("q", [T, DIM], BF16, kind="ExternalInput").ap()
    kv_d = nc.dram_tensor("kv", [T, DIM], BF16, kind="ExternalInput").ap()
    wq_d = nc.dram_tensor("wq", [DIM, DIM], BF16, kind="ExternalInput").ap()
    wkv_d = nc.dram_tensor("wkv", [DIM, 2 * DIM], BF16, kind="ExternalInput").ap()
    wo_d = nc.dram_tensor("wo", [DIM, DIM], BF16, kind="ExternalInput").ap()
    mask_d = nc.dram_tensor("maskneg", [TT, 4 * TT], F32, kind="ExternalInput").ap()
    bq_d = bkv_d = None
    if with_bias_q:
        bq_d = nc.dram_tensor("bq", [1, DIM], BF16, kind="ExternalInput").ap()
    if with_bias_kv:
        bkv_d = nc.dram_tensor("bkv", [1, 2 * DIM], BF16, kind="ExternalInput").ap()
    out_d = nc.dram_tensor("out", [T, DIM], BF16, kind="ExternalOutput").ap()

    NCH = T // TC  # chunks

    with tile.TileContext(nc) as tc, ExitStack() as ctx:
        # ---------------- static SBUF ----------------
        singles = ctx.enter_context(tc.tile_pool(name="singles", bufs=1))
        ident = singles.tile([128, 128], BF16)
        make_identity(nc, ident[:])
        maskneg = singles.tile([TT, 4 * TT], F32)
        nc.sync.dma_start(maskneg[:], mask_d)

        wq_sb = singles.tile([128, KT_F, DIM], BF16)
        wkv_sb = singles.tile([128, KT_F, 2 * DIM], BF16)
        wo_sb = singles.tile([128, KT_F, DIM], BF16)
        for k in range(KT_F):
            nc.sync.dma_start(wq_sb[:, k, :], wq_d[k * 128:(k + 1) * 128, :])
            nc.sync.dma_start(wkv_sb[:, k, :], wkv_d[k * 128:(k + 1) * 128, :])
            nc.sync.dma_start(wo_sb[:, k, :], wo_d[k * 128:(k + 1) * 128, :])
        if with_bias_q or with_bias_kv:
            ones_row = singles.tile([1, TC], BF16)
            nc.vector.memset(ones_row[:], 1.0)
        if with_bias_q:
            bq_sb = singles.tile([1, DIM], BF16)
            nc.sync.dma_start(bq_sb[:], bq_d)
        if with_bias_kv:
            bkv_sb = singles.tile([1, 2 * DIM], BF16)
            nc.sync.dma_start(bkv_sb[:], bkv_d)

        # ---------------- rotating pools ----------------
        xraw_p = ctx.enter_context(tc.tile_pool(name="xraw", bufs=10))
        st_p = ctx.enter_context(tc.tile_pool(name="stats", bufs=2))
        feat_p = ctx.enter_context(tc.tile_pool(name="feat", bufs=2))
        qkv_p = ctx.enter_context(tc.tile_pool(name="qkv", bufs=2))
        ctxT_p = ctx.enter_context(tc.tile_pool(name="ctxT", bufs=2))
        sm_p = ctx.enter_context(tc.tile_pool(name="smax", bufs=3))
        lv_p = ctx.enter_context(tc.tile_pool(name="lv", bufs=3))
        osb_p = ctx.enter_context(tc.tile_pool(name="osb", bufs=3))

        ps_proj = ctx.enter_context(tc.tile_pool(name="ps_proj", bufs=2, space="PSUM"))
        ps_o = ctx.enter_context(tc.tile_pool(name="ps_o", bufs=2, space="PSUM"))
        ps_tr = ctx.enter_context(tc.tile_pool(name="ps_tr", bufs=2, space="PSUM"))
        ps_at = ctx.enter_context(tc.tile_pool(name="ps_at", bufs=2, space="PSUM"))

        for c in range(NCH):
            # ---------- stage A: load + LN stats --------------------------
            xs = []  # 8 tiles: (q tile0, kv tile0, q tile1, kv tile1, ...)
            mv8 = st_p.tile([128, 8, 2], F32, tag="mv8")
            for it in range(TPC):
                tok0 = c * TC + it * TT
                for j, src in enumerate((q_d, kv_d)):
                    x = xraw_p.tile([128, DIM], BF16, tag="raw")
                    nc.sync.dma_start(x[:], src[tok0:tok0 + TT, :])
                    stats = st_p.tile([128, 2, 6], F32, tag="bn", bufs=8)
                    xg = x.rearrange("p (n f) -> p n f", n=2)
                    for i in range(2):
                        nc.vector.bn_stats(out=stats[:, i, :], in_=xg[:, i, :])
                    nc.vector.bn_aggr(out=mv8[:, it * 2 + j, :], in_=stats[:])
                    xs.append(x)

            # rstd for all 8 tiles: y = 1/sqrt(var + eps) via bit trick + NR
            rst = st_p.tile([128, 8], F32, tag="rst")
            tmp = st_p.tile([128, 8], F32, tag="rtmp")
            # rst = var + eps
            nc.vector.tensor_scalar(out=rst[:], in0=mv8[:, :, 1], scalar1=1e-5,
                                    scalar2=None, op0=mybir.AluOpType.add)
            # tmp_i = MAGIC - (bitcast(rst) >> 1)
            nc.vector.tensor_scalar(out=tmp[:].bitcast(I32),
                                    in0=rst[:].bitcast(I32),
                                    scalar1=1, scalar2=None,
                                    op0=mybir.AluOpType.arith_shift_right)
            nc.vector.tensor_scalar(out=tmp[:].bitcast(I32),
                                    in0=tmp[:].bitcast(I32),
                                    scalar1=-1, scalar2=MAGIC,
                                    op0=mybir.AluOpType.mult,
                                    op1=mybir.AluOpType.add)
            # two Newton iterations: y <- y * (1.5 - 0.5 * v * y * y)
            t2 = st_p.tile([128, 8], F32, tag="rt2")
            for _ in range(2):
                nc.vector.tensor_tensor(out=t2[:], in0=tmp[:], in1=tmp[:],
                                        op=mybir.AluOpType.mult)
                nc.vector.tensor_tensor(out=t2[:], in0=t2[:], in1=rst[:],
                                        op=mybir.AluOpType.mult)
                nc.vector.tensor_scalar(out=t2[:], in0=t2[:],
                                        scalar1=-0.5, scalar2=1.5,
                                        op0=mybir.AluOpType.mult,
                                        op1=mybir.AluOpType.add)
                nc.vector.tensor_tensor(out=tmp[:], in0=tmp[:], in1=t2[:],
                                        op=mybir.AluOpType.mult)

            # LN apply (in place) + PE transpose to feature-major
            qnT = feat_p.tile([128, KT_F, TC], BF16, tag="qnT")
            kvnT = feat_p.tile([128, KT_F, TC], BF16, tag="kvnT")
            for it in range(TPC):
                for j, dstT in enumerate((qnT, kvnT)):
                    idx = it * 2 + j
                    x = xs[idx]
                    nc.vector.tensor_scalar(out=x[:], in0=x[:],
                                            scalar1=mv8[:, idx, 0:1],
                                            scalar2=tmp[:, idx:idx + 1],
                                            op0=mybir.AluOpType.subtract,
                                            op1=mybir.AluOpType.mult)
                    for fg in range(2):  # two groups of 4 transposes
                        tp = ps_tr.tile([128, 4, 128], BF16, tag="tr")
                        for f4 in range(4):
                            f = fg * 4 + f4
                            nc.tensor.transpose(
                                tp[:, f4, :], x[:, f * 128:(f + 1) * 128],
                                ident[:])
                        # psum [128,4,128] -> dstT[:, 4fg:4fg+4, it*TT:+TT]
                        nc.scalar.copy(
                            out=dstT[:, fg * 4:(fg + 1) * 4,
                                     it * TT:(it + 1) * TT],
                            in_=tp[:])

            # ---------- stage B: Q/K/V projections ------------------------
            # head-major-within-subtile SBUF layout: [128, sub, head, 16]
            QT = qkv_p.tile([128, SPC, HEADS, TS], BF16, tag="QT")
            KT = qkv_p.tile([128, SPC, HEADS, TS], BF16, tag="KT")
            VT = qkv_p.tile([128, SPC, HEADS, TS], BF16, tag="VT")
            for m in range(HEADS):
                ps = ps_proj.tile([128, TC], F32, tag="mm")
                for k in range(KT_F):
                    nc.tensor.matmul(
                        ps[:], wq_sb[:, k, m * 128:(m + 1) * 128],
                        qnT[:, k, :], start=(k == 0),
                        stop=(k == KT_F - 1 and not with_bias_q))
                if with_bias_q:
                    nc.tensor.matmul(
                        ps[:], bq_sb[:, m * 128:(m + 1) * 128],
                        ones_row[:], start=False, stop=True)
                nc.vector.tensor_copy(
                    out=QT[:, :, m, :],
                    in_=ps[:].rearrange("p (s t) -> p s t", t=TS))
            for m in range(2 * HEADS):
                ps = ps_proj.tile([128, TC], F32, tag="mm")
                for k in range(KT_F):
                    nc.tensor.matmul(
                        ps[:], wkv_sb[:, k, m * 128:(m + 1) * 128],
                        kvnT[:, k, :], start=(k == 0),
                        stop=(k == KT_F - 1 and not with_bias_kv))
                if with_bias_kv:
                    nc.tensor.matmul(
                        ps[:], bkv_sb[:, m * 128:(m + 1) * 128],
                        ones_row[:], start=False, stop=True)
                dst = KT if m < HEADS else VT
                nc.vector.tensor_copy(
                    out=dst[:, :, m % HEADS, :],
                    in_=ps[:].rearrange("p (s t) -> p s t", t=TS))

            # ---------- stage C: attention + O-projection per tile --------
            for it in range(TPC):
                tok0 = c * TC + it * TT
                ctxT = ctxT_p.tile([128, HEADS, TT], BF16, tag="ctxT")
                for hh in range(2):  # two half-tiles of 4 sub-tiles
                    s0 = it * SPT + hh * 4
                    # scores for 4 sub-tiles into one PSUM bank
                    sc4 = ps_at.tile([128, 4, 128], F32, tag="at")
                    for s4 in range(4):
                        s = s0 + s4
                        nc.tensor.matmul(sc4[:, s4, :],
                                         QT[:, s, :, :], KT[:, s, :, :],
                                         start=True, stop=True)
                    # junk entries -> -60 (block-diagonal mask), in place
                    nc.vector.tensor_tensor(out=sc4[:], in0=sc4[:],
                                            in1=maskneg[:].rearrange(
                                                "p (s t) -> p s t", t=128),
                                            op=mybir.AluOpType.add)
                    # batched exp -> e4 (bf16), then row sums + reciprocal
                    e4 = sm_p.tile([128, 4, 128], BF16, tag="e4")
                    nc.scalar.activation(out=e4[:], in_=sc4[:],
                                         func=mybir.ActivationFunctionType.Exp,
                                         scale=1.0)
                    z4 = st_p.tile([128, 4], F32, tag="z4", bufs=4)
                    nc.vector.tensor_reduce(out=z4[:], in_=e4[:],
                                            op=mybir.AluOpType.add,
                                            axis=mybir.AxisListType.X)
                    zr4 = st_p.tile([128, 4], F32, tag="zr4", bufs=4)
                    nc.vector.reciprocal(out=zr4[:], in_=z4[:])
                    # P = E / Z  (per-partition scalar per sub-tile)
                    p4 = sm_p.tile([128, 4, 128], BF16, tag="p4")
                    for s4 in range(4):
                        nc.vector.tensor_scalar(out=p4[:, s4, :],
                                                in0=e4[:, s4, :],
                                                scalar1=zr4[:, s4:s4 + 1],
                                                scalar2=None,
                                                op0=mybir.AluOpType.mult)
                    # L = P^T per sub-tile (block-diagonal)
                    pt4 = ps_at.tile([128, 4, 128], BF16, tag="at")
                    for s4 in range(4):
                        nc.tensor.transpose(pt4[:, s4, :], p4[:, s4, :],
                                            ident[:])
                    lb4 = lv_p.tile([128, 4, 128], BF16, tag="lb4")
                    nc.vector.tensor_copy(out=lb4[:], in_=pt4[:])
                    # vb = V^T slice transposed -> [(g,t), d]
                    vt4 = ps_at.tile([128, 4, 128], BF16, tag="at")
                    for s4 in range(4):
                        nc.tensor.transpose(vt4[:, s4, :],
                                            VT[:, s0 + s4, :, :], ident[:])
                    vb4 = lv_p.tile([128, 4, 128], BF16, tag="vb4")
                    nc.vector.tensor_copy(out=vb4[:], in_=vt4[:])
                    # ctx^T[d, (h,t)] = vb^T @ L
                    cx4 = ps_at.tile([128, 4, 128], F32, tag="at")
                    for s4 in range(4):
                        nc.tensor.matmul(cx4[:, s4, :], vb4[:, s4, :],
                                         lb4[:, s4, :],
                                         start=True, stop=True)
                    # psum cols (s4, h, t) -> ctxT[:, h, (s0%SPT+s4)*16 + t]
                    st_off = hh * 4 * TS
                    ctx_dst = bass.AP(
                        tensor=ctxT.tensor,
                        offset=ctxT.offset + st_off,
                        ap=[ctxT.ap[0], [TS, 4], [TT, HEADS], [1, TS]])
                    nc.scalar.copy(
                        out=ctx_dst,
                        in_=cx4[:].rearrange("p s (h t) -> p s h t", t=TS))

                # O-projection, token-major
                for half in range(2):
                    po = ps_o.tile([128, 512], F32, tag="po")
                    for h in range(HEADS):
                        nc.tensor.matmul(
                            po[:], ctxT[:, h, :],
                            wo_sb[:, h, half * 512:(half + 1) * 512],
                            start=(h == 0), stop=(h == HEADS - 1))
                    osb = osb_p.tile([128, 512], BF16, tag="osb")
                    nc.scalar.copy(out=osb[:], in_=po[:])
                    nc.sync.dma_start(
                        out_d[tok0:tok0 + TT, half * 512:(half + 1) * 512],
                        osb[:])

    nc.finalize()
    return nc


def _host_maskneg():
    m = np.full((TT, TT), -60.0, np.float32)
    p = np.arange(TT)
    m[(p[:, None] % TS) == (p[None, :] % TS)] = 0.0
    return np.tile(m, (1, 4)).copy()


def kernel(q, kv, gamma_m, beta_m, gamma_l, beta_l, Wq, Wkv, Wo):
    q = np.asarray(q, np.float32)
    kv = np.asarray(kv, np.float32)
    bs, patch, _ = q.shape
    T_total = bs * patch
    T_core = T_total // NCORES

    scale = DHEAD ** (-0.5)
    # fold LN gamma into the projection weights, beta into bias vectors
    wq_eff = (np.asarray(Wq, np.float32) * np.asarray(gamma_m, np.float32)[None, :]) * scale
    bq = (np.asarray(Wq, np.float32) @ np.asarray(beta_m, np.float32)) * scale
    wkv_eff = np.asarray(Wkv, np.float32) * np.asarray(gamma_l, np.float32)[None, :]
    bkv = np.asarray(Wkv, np.float32) @ np.asarray(beta_l, np.float32)
    with_bias_q = bool(np.any(bq != 0.0))
    with_bias_kv = bool(np.any(bkv != 0.0))

    bf16 = ml_dtypes.bfloat16
    # kernel weight layout: [in, out]
    wq_t = np.ascontiguousarray(wq_eff.T).astype(bf16)
    wkv_t = np.ascontiguousarray(wkv_eff.T).astype(bf16)
    wo_t = np.ascontiguousarray(np.asarray(Wo, np.float32).T).astype(bf16)
    maskneg = _host_maskneg()

    nc = build_nc(T_core, with_bias_q, with_bias_kv)

    qf = np.ascontiguousarray(q.reshape(T_total, DIM)).astype(bf16)
    kvf = np.ascontiguousarray(kv.reshape(T_total, DIM)).astype(bf16)
    in_maps = []
    for i in range(NCORES):
        m = {
            "q": np.ascontiguousarray(qf[i * T_core:(i + 1) * T_core]),
            "kv": np.ascontiguousarray(kvf[i * T_core:(i + 1) * T_core]),
            "wq": wq_t, "wkv": wkv_t, "wo": wo_t, "maskneg": maskneg,
        }
        if with_bias_q:
            m["bq"] = bq.reshape(1, DIM).astype(bf16)
        if with_bias_kv:
            m["bkv"] = bkv.reshape(1, 2 * DIM).astype(bf16)
        in_maps.append(m)

    res = run_bass_kernel_spmd(nc, in_maps, list(range(NCORES)))
    global LAST_RESULTS
    LAST_RESULTS = res
    out = np.concatenate(
        [np.asarray(res.results[i]["out"]).astype(np.float32)
         for i in range(NCORES)], axis=0)
    return out.reshape(bs, patch, DIM)


LAST_RESULTS = None


# revision 21
# speedup vs baseline: 1.0565x; 1.0565x over previous
"""Trainium2 Bass kernel for nn_MultiHeadAttention_7834020348049.

Reference computation (per token, no cross-token interaction):
    qn  = LayerNorm(q) * gamma_m + beta_m
    kvn = LayerNorm(kv) * gamma_l + beta_l
    Q = qn @ Wq.T ; K,V = split(kvn @ Wkv.T)
    per token: scores[h,g] = Q[h,:] . K[g,:] / sqrt(128)  (8x8 over heads)
    ctx[h,:] = softmax_g(scores) @ V
    out = ctx @ Wo.T

Sharding: pure data-parallel over the 16*2048 = 32768 tokens -> 4096/core.

Per-core pipeline (bf16 matmuls, fp32 PSUM):
  token-major LN (bn_stats/bn_aggr, rsqrt via bit-trick+Newton on DVE)
  -> PE transpose to feature-major qn^T / kvn^T (bf16)
  -> Q/K/V projections, weights stationary, N=512 moving; Q/V stored
     per-head plain (contiguous copies), K packed per-16-token-subtile
  -> per 16-token sub-tile (partition index p = h*16 + t):
     scores matmul S[(h,t),(g,t')] (4 per PSUM bank), -60 off-block mask
     accumulated via one extra matmul (maskT stationary, tiled identity
     moving), batched exp on ACT, row sums + recip + P=E/Z on DVE,
     PE transpose of P -> L, PE transpose of V slice -> vb,
     ctx^T = vb.T @ L
  -> token-major O-projection (ctx head-slices stationary), bf16 output.
ACT engine only ever runs Exp and Copy (single activation table set).
"""
import sys, os
sys.path.insert(0, "/opt/trn_rl_repo")
os.environ.setdefault("JAX_PLATFORMS", "cpu")

from contextlib import ExitStack
import numpy as np
import ml_dtypes

import concourse.bass as bass
import concourse.bacc as bacc
import concourse.tile as tile
from concourse import mybir
from concourse.masks import make_identity
from concourse.bass_utils import run_bass_kernel_spmd

F32 = mybir.dt.float32
BF16 = mybir.dt.bfloat16
I32 = mybir.dt.int32

DIM = 1024
HEADS = 8
DHEAD = 128
NCORES = 8

TT = 128          # tokens per tile (partition dim)
TC = 512          # tokens per chunk (projection moving-dim)
TS = 16           # tokens per attention sub-tile
TPC = TC // TT    # tiles per chunk (4)
SPT = TT // TS    # sub-tiles per tile (8)
SPC = TC // TS    # sub-tiles per chunk (32)
KT_F = DIM // 128 # k-tiles for the 1024-feature contraction (8)

MAGIC = 0x5F3759DF  # fast inverse sqrt seed


def build_nc(T, with_bias_q=False, with_bias_kv=False):
    """Build the single-core Bass program for T tokens."""
    nc = bacc.Bacc(trn_type="TRN2", target_bir_lowering=False)

    q_d = nc.dram_tensor("q", [T, DIM], BF16, kind="ExternalInput").ap()
    kv_d = nc.dram_tensor("kv", [T, DIM], BF16, kind="ExternalInput").ap()
    wq_d = nc.dram_tensor("wq", [DIM, DIM], BF16, kind="ExternalInput").ap()
    wkv_d = nc.dram_tensor("wkv", [DIM, 2 * DIM], BF16, kind="ExternalInput").ap()
    wo_d = nc.dram_tensor("wo", [DIM, DIM], BF16, kind="ExternalInput").ap()
    mask_d = nc.dram_tensor("maskneg", [TT, TT], BF16, kind="ExternalInput").ap()
    id4_d = nc.dram_tensor("ident4", [TT, 4 * TT], BF16, kind="ExternalInput").ap()
    bq_d = bkv_d = None
    if with_bias_q:
        bq_d = nc.dram_tensor("bq", [1, DIM], BF16, kind="ExternalInput").ap()
    if with_bias_kv:
        bkv_d = nc.dram_tensor("bkv", [1, 2 * DIM], BF16, kind="ExternalInput").ap()
    out_d = nc.dram_tensor("out", [T, DIM], BF16, kind="ExternalOutput").ap()

    NCH = T // TC  # chunks

    with tile.TileContext(nc) as tc, ExitStack() as ctx:
        # ---------------- static SBUF ----------------
        singles = ctx.enter_context(tc.tile_pool(name="singles", bufs=1))
        ident = singles.tile([128, 128], BF16)
        make_identity(nc, ident[:])
        maskneg = singles.tile([TT, TT], BF16)
        nc.sync.dma_start(maskneg[:], mask_d)
        ident4 = singles.tile([TT, 4 * TT], BF16)
        nc.sync.dma_start(ident4[:], id4_d)

        # rotating pools (x DMAs for chunk 0 prefetched before weights)
        xc_p = ctx.enter_context(tc.tile_pool(name="xc", bufs=2))

        def load_chunk(c):
            """one DMA per tensor per chunk: [128, TPC, DIM] bf16"""
            tiles = []
            for nm, src in (("xq", q_d), ("xkv", kv_d)):
                xch = xc_p.tile([128, TPC, DIM], BF16, tag=nm, name=nm)
                src_ap = bass.AP(
                    tensor=src.tensor,
                    offset=src.offset + c * TC * DIM,
                    ap=[[DIM, 128], [TT * DIM, TPC], [1, DIM]])
                nc.sync.dma_start(xch[:], src_ap)
                tiles.append(xch)
            return tiles

        pre0 = load_chunk(0)

        wq_sb = singles.tile([128, KT_F, DIM], BF16)
        wkv_sb = singles.tile([128, KT_F, 2 * DIM], BF16)
        wo_sb = singles.tile([128, KT_F, DIM], BF16)
        for k in range(KT_F):
            nc.sync.dma_start(wq_sb[:, k, :], wq_d[k * 128:(k + 1) * 128, :])
        for k in range(KT_F):
            nc.sync.dma_start(wkv_sb[:, k, :], wkv_d[k * 128:(k + 1) * 128, :])
        for k in range(KT_F):
            nc.sync.dma_start(wo_sb[:, k, :], wo_d[k * 128:(k + 1) * 128, :])
        if with_bias_q or with_bias_kv:
            ones_row = singles.tile([1, TC], BF16)
            nc.vector.memset(ones_row[:], 1.0)
        if with_bias_q:
            bq_sb = singles.tile([1, DIM], BF16)
            nc.sync.dma_start(bq_sb[:], bq_d)
        if with_bias_kv:
            bkv_sb = singles.tile([1, 2 * DIM], BF16)
            nc.sync.dma_start(bkv_sb[:], bkv_d)

        st_p = ctx.enter_context(tc.tile_pool(name="stats", bufs=2))
        feat_p = ctx.enter_context(tc.tile_pool(name="feat", bufs=2))
        qkv_p = ctx.enter_context(tc.tile_pool(name="qkv", bufs=2))
        ctxT_p = ctx.enter_context(tc.tile_pool(name="ctxT", bufs=2))
        sm_p = ctx.enter_context(tc.tile_pool(name="smax", bufs=4))
        lv_p = ctx.enter_context(tc.tile_pool(name="lv", bufs=4))
        osb_p = ctx.enter_context(tc.tile_pool(name="osb", bufs=2))

        ps_proj = ctx.enter_context(tc.tile_pool(name="ps_proj", bufs=2, space="PSUM"))
        ps_o = ctx.enter_context(tc.tile_pool(name="ps_o", bufs=2, space="PSUM"))
        ps_tr = ctx.enter_context(tc.tile_pool(name="ps_tr", bufs=2, space="PSUM"))
        ps_at = ctx.enter_context(tc.tile_pool(name="ps_at", bufs=2, space="PSUM"))

        for c in range(NCH):
            # ---------- stage A: load + LN stats --------------------------
            xq, xkv = pre0 if c == 0 else load_chunk(c)
            mv8 = st_p.tile([128, 8, 2], F32, tag="mv8")
            for it in range(TPC):
                for j, xch in enumerate((xq, xkv)):
                    stats = st_p.tile([128, 2, 6], F32, tag="bn", bufs=8)
                    for i in range(2):
                        nc.vector.bn_stats(
                            out=stats[:, i, :],
                            in_=xch[:, it, i * 512:(i + 1) * 512])
                    nc.vector.bn_aggr(out=mv8[:, it * 2 + j, :], in_=stats[:])

            # rstd for all 8 tiles: y = 1/sqrt(var + eps) via bit trick + NR
            rst = st_p.tile([128, 8], F32, tag="rst")
            tmp = st_p.tile([128, 8], F32, tag="rtmp")
            nc.vector.tensor_scalar(out=rst[:], in0=mv8[:, :, 1], scalar1=1e-5,
                                    scalar2=None, op0=mybir.AluOpType.add)
            nc.vector.tensor_scalar(out=tmp[:].bitcast(I32),
                                    in0=rst[:].bitcast(I32),
                                    scalar1=1, scalar2=None,
                                    op0=mybir.AluOpType.arith_shift_right)
            nc.vector.tensor_scalar(out=tmp[:].bitcast(I32),
                                    in0=tmp[:].bitcast(I32),
                                    scalar1=-1, scalar2=MAGIC,
                                    op0=mybir.AluOpType.mult,
                                    op1=mybir.AluOpType.add)
            t2 = st_p.tile([128, 8], F32, tag="rt2")
            for _ in range(2):
                nc.vector.tensor_tensor(out=t2[:], in0=tmp[:], in1=tmp[:],
                                        op=mybir.AluOpType.mult)
                nc.vector.tensor_tensor(out=t2[:], in0=t2[:], in1=rst[:],
                                        op=mybir.AluOpType.mult)
                nc.vector.tensor_scalar(out=t2[:], in0=t2[:],
                                        scalar1=-0.5, scalar2=1.5,
                                        op0=mybir.AluOpType.mult,
                                        op1=mybir.AluOpType.add)
                nc.vector.tensor_tensor(out=tmp[:], in0=tmp[:], in1=t2[:],
                                        op=mybir.AluOpType.mult)

            # LN apply (in place) + PE transpose to feature-major
            qnT = feat_p.tile([128, KT_F, TC], BF16, tag="qnT")
            kvnT = feat_p.tile([128, KT_F, TC], BF16, tag="kvnT")
            for it in range(TPC):
                for j, (xch, dstT) in enumerate(((xq, qnT), (xkv, kvnT))):
                    idx = it * 2 + j
                    nc.vector.tensor_scalar(out=xch[:, it, :], in0=xch[:, it, :],
                                            scalar1=mv8[:, idx, 0:1],
                                            scalar2=tmp[:, idx:idx + 1],
                                            op0=mybir.AluOpType.subtract,
                                            op1=mybir.AluOpType.mult)
                    for fg in range(2):  # two groups of 4 transposes
                        tp = ps_tr.tile([128, 4, 128], BF16, tag="tr")
                        for f4 in range(4):
                            f = fg * 4 + f4
                            nc.tensor.transpose(
                                tp[:, f4, :],
                                xch[:, it, f * 128:(f + 1) * 128],
                                ident[:])
                        nc.vector.tensor_copy(
                            out=dstT[:, fg * 4:(fg + 1) * 4,
                                     it * TT:(it + 1) * TT],
                            in_=tp[:])

            # ---------- stage B: Q/K/V projections ------------------------
            # all packed per-sub-tile [128, sub, head, 16]: matmul operand
            # APs (stationary and moving) must be flat single-free-dim.
            QT = qkv_p.tile([128, SPC, HEADS, TS], BF16, tag="QT")
            KT = qkv_p.tile([128, SPC, HEADS, TS], BF16, tag="KT")
            VT = qkv_p.tile([128, SPC, HEADS, TS], BF16, tag="VT")
            for m in range(HEADS):
                ps = ps_proj.tile([128, TC], F32, tag="mm")
                for k in range(KT_F):
                    nc.tensor.matmul(
                        ps[:], wq_sb[:, k, m * 128:(m + 1) * 128],
                        qnT[:, k, :], start=(k == 0),
                        stop=(k == KT_F - 1 and not with_bias_q))
                if with_bias_q:
                    nc.tensor.matmul(
                        ps[:], bq_sb[:, m * 128:(m + 1) * 128],
                        ones_row[:], start=False, stop=True)
                nc.vector.tensor_copy(
                    out=QT[:, :, m, :],
                    in_=ps[:].rearrange("p (s t) -> p s t", t=TS))
            for m in range(2 * HEADS):
                ps = ps_proj.tile([128, TC], F32, tag="mm")
                for k in range(KT_F):
                    nc.tensor.matmul(
                        ps[:], wkv_sb[:, k, m * 128:(m + 1) * 128],
                        kvnT[:, k, :], start=(k == 0),
                        stop=(k == KT_F - 1 and not with_bias_kv))
                if with_bias_kv:
                    nc.tensor.matmul(
                        ps[:], bkv_sb[:, m * 128:(m + 1) * 128],
                        ones_row[:], start=False, stop=True)
                dst = KT if m < HEADS else VT
                nc.vector.tensor_copy(
                    out=dst[:, :, m % HEADS, :],
                    in_=ps[:].rearrange("p (s t) -> p s t", t=TS))

            # ---------- stage C: attention + O-projection per tile --------
            for it in range(TPC):
                tok0 = c * TC + it * TT
                ctxT = ctxT_p.tile([128, HEADS, TT], BF16, tag="ctxT")
                for hh in range(2):  # two half-tiles of 4 sub-tiles
                    s0 = it * SPT + hh * 4
                    # scores for 4 sub-tiles into one PSUM bank, then the
                    # block-diagonal -60 mask via one accumulated matmul
                    sc4 = ps_at.tile([128, 4, 128], F32, tag="at")
                    for s4 in range(4):
                        s = s0 + s4
                        nc.tensor.matmul(sc4[:, s4, :],
                                         QT[:, s, :, :], KT[:, s, :, :],
                                         start=True, stop=True)
                    for s4 in range(4):
                        nc.vector.tensor_tensor(out=sc4[:, s4, :],
                                                in0=sc4[:, s4, :],
                                                in1=maskneg[:],
                                                op=mybir.AluOpType.add)
                    # batched exp -> e4 (bf16), then row sums + reciprocal
                    e4 = sm_p.tile([128, 4, 128], BF16, tag="e4")
                    nc.scalar.activation(out=e4[:], in_=sc4[:],
                                         func=mybir.ActivationFunctionType.Exp,
                                         scale=1.0)
                    z4 = st_p.tile([128, 4], F32, tag="z4", bufs=4)
                    nc.vector.tensor_reduce(out=z4[:], in_=e4[:],
                                            op=mybir.AluOpType.add,
                                            axis=mybir.AxisListType.X)
                    zr4 = st_p.tile([128, 4], F32, tag="zr4", bufs=4)
                    nc.vector.reciprocal(out=zr4[:], in_=z4[:])
                    # P = E / Z  (per-partition scalar per sub-tile)
                    p4 = sm_p.tile([128, 4, 128], BF16, tag="p4")
                    for s4 in range(4):
                        nc.vector.tensor_scalar(out=p4[:, s4, :],
                                                in0=e4[:, s4, :],
                                                scalar1=zr4[:, s4:s4 + 1],
                                                scalar2=None,
                                                op0=mybir.AluOpType.mult)
                    # L = P^T per sub-tile (block-diagonal)
                    pt4 = ps_at.tile([128, 4, 128], BF16, tag="at")
                    for s4 in range(4):
                        nc.tensor.transpose(pt4[:, s4, :], p4[:, s4, :],
                                            ident[:])
                    lb4 = lv_p.tile([128, 4, 128], BF16, tag="lb4")
                    nc.scalar.copy(out=lb4[:], in_=pt4[:])
                    # vb = V^T slice transposed -> [(g,t), d]
                    vt4 = ps_at.tile([128, 4, 128], BF16, tag="at")
                    for s4 in range(4):
                        nc.tensor.transpose(vt4[:, s4, :],
                                            VT[:, s0 + s4, :, :], ident[:])
                    vb4 = lv_p.tile([128, 4, 128], BF16, tag="vb4")
                    nc.scalar.copy(out=vb4[:], in_=vt4[:])
                    # ctx^T[d, (h,t)] = vb^T @ L
                    cx4 = ps_at.tile([128, 4, 128], F32, tag="at")
                    for s4 in range(4):
                        nc.tensor.matmul(cx4[:, s4, :], vb4[:, s4, :],
                                         lb4[:, s4, :],
                                         start=True, stop=True)
                    # psum cols (s4, h, t) -> ctxT[:, h, (hh*4+s4)*16 + t]
                    st_off = hh * 4 * TS
                    ctx_dst = bass.AP(
                        tensor=ctxT.tensor,
                        offset=ctxT.offset + st_off,
                        ap=[ctxT.ap[0], [TS, 4], [TT, HEADS], [1, TS]])
                    nc.scalar.copy(
                        out=ctx_dst,
                        in_=cx4[:].rearrange("p s (h t) -> p s h t", t=TS))

                # O-projection, token-major
                osb = osb_p.tile([128, DIM], BF16, tag="osb")
                for half in range(2):
                    po = ps_o.tile([128, 512], F32, tag="po")
                    for h in range(HEADS):
                        nc.tensor.matmul(
                            po[:], ctxT[:, h, :],
                            wo_sb[:, h, half * 512:(half + 1) * 512],
                            start=(h == 0), stop=(h == HEADS - 1))
                    nc.scalar.copy(out=osb[:, half * 512:(half + 1) * 512],
                                   in_=po[:])
                nc.sync.dma_start(out_d[tok0:tok0 + TT, :], osb[:])

    nc.finalize()
    return nc


def _host_maskneg():
    m = np.full((TT, TT), -60.0, np.float32)
    p = np.arange(TT)
    m[(p[:, None] % TS) == (p[None, :] % TS)] = 0.0
    return m


def kernel(q, kv, gamma_m, beta_m, gamma_l, beta_l, Wq, Wkv, Wo):
    q = np.asarray(q, np.float32)
    kv = np.asarray(kv, np.float32)
    bs, patch, _ = q.shape
    T_total = bs * patch
    T_core = T_total // NCORES

    scale = DHEAD ** (-0.5)
    # fold LN gamma into the projection weights, beta into bias vectors
    wq_eff = (np.asarray(Wq, np.float32) * np.asarray(gamma_m, np.float32)[None, :]) * scale
    bq = (np.asarray(Wq, np.float32) @ np.asarray(beta_m, np.float32)) * scale
    wkv_eff = np.asarray(Wkv, np.float32) * np.asarray(gamma_l, np.float32)[None, :]
    bkv = np.asarray(Wkv, np.float32) @ np.asarray(beta_l, np.float32)
    with_bias_q = bool(np.any(bq != 0.0))
    with_bias_kv = bool(np.any(bkv != 0.0))

    bf16 = ml_dtypes.bfloat16
    # kernel weight layout: [in, out]
    wq_t = np.ascontiguousarray(wq_eff.T).astype(bf16)
    wkv_t = np.ascontiguousarray(wkv_eff.T).astype(bf16)
    wo_t = np.ascontiguousarray(np.asarray(Wo, np.float32).T).astype(bf16)
    maskneg = _host_maskneg().astype(bf16)
    ident4 = np.tile(np.eye(TT, dtype=np.float32), (1, 4)).astype(bf16)

    nc = build_nc(T_core, with_bias_q, with_bias_kv)

    qf = np.ascontiguousarray(q.reshape(T_total, DIM)).astype(bf16)
    kvf = np.ascontiguousarray(kv.reshape(T_total, DIM)).astype(bf16)
    in_maps = []
    for i in range(NCORES):
        m = {
            "q": np.ascontiguousarray(qf[i * T_core:(i + 1) * T_core]),
            "kv": np.ascontiguousarray(kvf[i * T_core:(i + 1) * T_core]),
            "wq": wq_t, "wkv": wkv_t, "wo": wo_t, "maskneg": maskneg,
            "ident4": ident4,
        }
        if with_bias_q:
            m["bq"] = bq.reshape(1, DIM).astype(bf16)
        if with_bias_kv:
            m["bkv"] = bkv.reshape(1, 2 * DIM).astype(bf16)
        in_maps.append(m)

    res = run_bass_kernel_spmd(nc, in_maps, list(range(NCORES)))
    global LAST_RESULTS
    LAST_RESULTS = res
    out = np.concatenate(
        [np.asarray(res.results[i]["out"]).astype(np.float32)
         for i in range(NCORES)], axis=0)
    return out.reshape(bs, patch, DIM)


LAST_RESULTS = None
